# revision 1
# baseline (speedup 1.0000x reference)
"""Exponential smoothing (per-channel EMA over time) on 8 Trainium2 cores.

  s_0 = x_0 ; s_t = a * x_t + (1 - a) * s_{t-1},  a = sigmoid(alpha)  (per channel)

Full shapes: x (16, 4096, 512) f32, alpha (1, 1, 512) f32 -> out (16, 4096, 512).

Sharding: data-parallel over batch B (16 -> 2 per core); alpha replicated.
Per core the kernel:
  1. DMA-loads x in native layout (t on partitions, d on free) — contiguous
     2 KB per partition, full HBM line rate.
  2. Transposes 128x128 blocks on the tensor engine (PSUM out) so time lands
     on the free axis.
  3. Evacuates PSUM via the scalar engine with the per-channel scale `a`
     fused in (u = a * x^T).
  4. Runs the hardware scan (TensorTensorScanArith) on the vector engine:
     state = w * state + u with w = 1 - a = sigmoid(-alpha), chained across
     time chunks via `initial`. Chunk 0 uses initial = x_0 (raw), which makes
     s_0 = w*x_0 + a*x_0 = x_0 exactly.
  5. Transposes back on the tensor engine, evacuates, DMA-stores.
"""

from contextlib import ExitStack

import numpy as np

import concourse.bass as bass
import concourse.tile as tile
from concourse import bacc, mybir
from concourse.bass_utils import run_bass_kernel_spmd
from concourse.masks import make_identity

B, T, D = 16, 4096, 512
NCORES = 8
BL = B // NCORES  # batches per core
P = 128           # partitions
TC = 512          # time-chunk processed per pipeline iteration
NTC = T // TC     # time chunks
ND = D // P       # channel chunks of 128
NK = TC // P      # 128-row sub-chunks per time chunk

FP32 = mybir.dt.float32


def build_program(bl: int = BL, t: int = T, evac2_dve_ks: tuple = (3,)) -> bacc.Bacc:
    """Build the per-core Bass program (same NEFF for all 8 cores).

    evac2_dve_ks: which output-transpose sub-chunks get evacuated on the
    vector engine instead of the scalar engine (load balancing).
    """
    ntc = t // TC
    nc = bacc.Bacc(
        "TRN2",
        target_bir_lowering=False,
        debug=False,
        enable_asserts=False,
        num_devices=NCORES,
    )
    x = nc.dram_tensor("x", (bl, t, D), FP32, kind="ExternalInput").ap()
    alpha = nc.dram_tensor("alpha", (1, 1, D), FP32, kind="ExternalInput").ap()
    y = nc.dram_tensor("y", (bl, t, D), FP32, kind="ExternalOutput").ap()

    with tile.TileContext(nc) as tc, ExitStack() as ctx:
        const_pool = ctx.enter_context(tc.tile_pool(name="const", bufs=1))
        xn_pool = ctx.enter_context(tc.tile_pool(name="xn", bufs=3))
        pin_pool = ctx.enter_context(tc.tile_pool(name="pin", bufs=4, space="PSUM"))
        pout_pool = ctx.enter_context(tc.tile_pool(name="pout", bufs=4, space="PSUM"))
        u_pool = ctx.enter_context(tc.tile_pool(name="u", bufs=8))
        s_pool = ctx.enter_context(tc.tile_pool(name="s", bufs=12))
        y_pool = ctx.enter_context(tc.tile_pool(name="y", bufs=3))

        ident = const_pool.tile([P, P], FP32)
        make_identity(nc, ident[:])

        # alpha (1,1,512) -> (128, ND) tile: channel d = j*128 + p
        alpha_sb = const_pool.tile([P, ND], FP32)
        nc.sync.dma_start(alpha_sb[:], alpha.rearrange("o u (j p) -> (o u p) j", p=P))
        a_sb = const_pool.tile([P, ND], FP32)  # a = sigmoid(alpha)
        nc.scalar.activation(a_sb[:], alpha_sb[:], mybir.ActivationFunctionType.Sigmoid)
        w_sb = const_pool.tile([P, ND], FP32)  # w = 1 - a = sigmoid(-alpha)
        nc.scalar.activation(
            w_sb[:], alpha_sb[:], mybir.ActivationFunctionType.Sigmoid, scale=-1.0
        )

        # Per-channel-chunk decay tiles broadcast along the time axis
        # (scan data0 must be a full [P, TC] operand).
        ones = const_pool.tile([P, TC], FP32)
        nc.vector.memset(ones[:], 1.0)
        wbs = []
        for j in range(ND):
            wt = const_pool.tile([P, TC], FP32, tag=f"wb{j}")
            nc.vector.tensor_scalar_mul(wt[:], ones[:], w_sb[:, j : j + 1])
            wbs.append(wt)

        for b in range(bl):
            s_prev = [None] * ND
            for tci in range(ntc):
                t0 = tci * TC
                # Load TC time rows in native layout: partition = t % 128,
                # free = (k, d). DRAM side is contiguous 2 KB per partition.
                xn = xn_pool.tile([P, NK, D], FP32, tag="xn")
                nc.sync.dma_start(
                    xn[:], x[b, t0 : t0 + TC, :].rearrange("(k p) d -> p k d", p=P)
                )

                # Transpose to (d-part, t-free), one PSUM bank per d-chunk.
                pins = []
                for j in range(ND):
                    pin = pin_pool.tile([P, TC], FP32, tag="pin")
                    for k in range(NK):
                        nc.tensor.transpose(
                            pin[:, k * P : (k + 1) * P],
                            xn[:, k, j * P : (j + 1) * P],
                            ident[:],
                        )
                    pins.append(pin)

                # u = a * x^T  (scalar engine, PSUM -> SBUF, scale fused)
                us = []
                for j in range(ND):
                    u = u_pool.tile([P, TC], FP32, tag="u")
                    nc.scalar.mul(u[:], pins[j][:], a_sb[:, j : j + 1])
                    us.append(u)

                # Hardware scan along the free (time) axis.
                ss = []
                for j in range(ND):
                    s = s_pool.tile([P, TC], FP32, tag="s")
                    init = (
                        pins[j][:, 0:1] if tci == 0 else s_prev[j][:, TC - 1 : TC]
                    )
                    nc.vector.tensor_tensor_scan(
                        s[:],
                        wbs[j][:],
                        us[j][:],
                        init,
                        mybir.AluOpType.mult,
                        mybir.AluOpType.add,
                    )
                    ss.append(s)
                s_prev = ss

                # Transpose back to native layout and store.
                yout = y_pool.tile([P, NK, D], FP32, tag="y")
                for k in range(NK):
                    pout = pout_pool.tile([P, D], FP32, tag="pout")
                    for j in range(ND):
                        nc.tensor.transpose(
                            pout[:, j * P : (j + 1) * P],
                            ss[j][:, k * P : (k + 1) * P],
                            ident[:],
                        )
                    if k in evac2_dve_ks:
                        nc.vector.tensor_copy(yout[:, k, :], pout[:])
                    else:
                        nc.scalar.copy(yout[:, k, :], pout[:])
                nc.scalar.dma_start(
                    y[b, t0 : t0 + TC, :].rearrange("(k p) d -> p k d", p=P), yout[:]
                )

    nc.compile()
    return nc


_prog = None


def kernel(x, alpha):
    global _prog
    if _prog is None:
        _prog = build_program()
    x = np.ascontiguousarray(np.asarray(x, dtype=np.float32))
    alpha = np.ascontiguousarray(np.asarray(alpha, dtype=np.float32))
    assert x.shape == (B, T, D) and alpha.shape == (1, 1, D)
    in_maps = [
        {"x": np.ascontiguousarray(x[i * BL : (i + 1) * BL]), "alpha": alpha}
        for i in range(NCORES)
    ]
    res = run_bass_kernel_spmd(_prog, in_maps, core_ids=list(range(NCORES)))
    return np.concatenate([r["y"] for r in res.results], axis=0)
